# revision 1
# baseline (speedup 1.0000x reference)
"""CondConv2D Trainium2 kernel: data-parallel over batch across 8 NeuronCores.

Per core (4 samples):
  1. alphas = softmax(cond @ alpha_w + alpha_b)              [tiny PE matmul + ACT/DVE]
  2. K_mix[b] = sum_e alphas[b,e] * expert_kernels[e]        [ACT mul + DVE scalar_tensor_tensor]
  3. conv2d(x[b], K_mix[b], SAME) + bias_mix[b]

Conv strategy (column-major shifted matmul):
  x[b] loads in natural [h, (w,c)] layout (contiguous 32KB/partition DMA); PE
  transposes of contiguous 2-column blocks build S[(half,c), pair q, padded-h]
  bf16 where pair q stacks column q-1 (partitions 0:63, conv tap kw=0) and
  column q (partitions 64:127, tap kw=1). Per group of 4 output columns, one
  PSUM bank [F, 4*128] accumulates: 3 K=128 matmuls (taps kw=0,1 for each kh
  via padded-h offsets, N=512) plus 3 K=64 matmuls for tap kw=2 (columns
  w0+1..w0+4 are the tops of pairs w0+2..w0+5, N=512). Mixed bias rides the
  PSUM evacuation (ACT Identity + per-partition bias). PE transposes [F,h] ->
  [h,F] into one PSUM bank per group -> single ACT evac -> contiguous
  512B-run DMA stores to HBM [B,H,W,F].
"""

import numpy as np

import concourse.bass as bass
import concourse.bacc as bacc
import concourse.mybir as mybir
import concourse.tile as tile
from concourse.bass_utils import run_bass_kernel_spmd
from concourse.masks import make_identity

B, H, W, Cin, E, F = 32, 128, 128, 64, 4, 128
KH = KW = 3
NCORES = 8
NB = B // NCORES  # 4 samples per core
CD = 64  # cond dim
HP = H + 2  # padded h size (130)
NQ = W + 2  # column pairs q=0..129; pair q: top=col q-1, bottom=col q

FP32 = mybir.dt.float32
BF16 = mybir.dt.bfloat16
AF = mybir.ActivationFunctionType
ALU = mybir.AluOpType

_cache = {}


def _build_nc():
    nc = bacc.Bacc(None)
    x_in = nc.dram_tensor("x", [NB, H, W, Cin], FP32, kind="ExternalInput")
    cond_in = nc.dram_tensor("cond", [NB, CD], FP32, kind="ExternalInput")
    aw_in = nc.dram_tensor("alpha_w", [CD, E], FP32, kind="ExternalInput")
    ab_in = nc.dram_tensor("alpha_b", [E], FP32, kind="ExternalInput")
    ek_in = nc.dram_tensor("expert_kernels", [E, KH, KW, Cin, F], FP32, kind="ExternalInput")
    eb_in = nc.dram_tensor("expert_bias", [E, F], FP32, kind="ExternalInput")
    out_t = nc.dram_tensor("out", [NB, H, W, F], FP32, kind="ExternalOutput")

    with tile.TileContext(nc) as tc:
        with (
            tc.tile_pool(name="const", bufs=1) as const_pool,
            tc.tile_pool(name="ek", bufs=1) as ek_pool,
            tc.tile_pool(name="mix", bufs=2) as mix_pool,
            tc.tile_pool(name="wts", bufs=2) as w_pool,
            tc.tile_pool(name="xin", bufs=2) as x_pool,
            tc.tile_pool(name="stk", bufs=2) as s_pool,
            tc.tile_pool(name="ev1", bufs=3) as sb1_pool,
            tc.tile_pool(name="ev2", bufs=3) as sb2_pool,
            tc.tile_pool(name="small", bufs=2) as small_pool,
            tc.tile_pool(name="dram", bufs=1, space="DRAM") as dram_pool,
            tc.tile_pool(name="pconv", bufs=3, space="PSUM") as pconv_pool,
            tc.tile_pool(name="ptin", bufs=2, space="PSUM") as ptin_pool,
            tc.tile_pool(name="ptout", bufs=2, space="PSUM") as ptout_pool,
            tc.tile_pool(name="psmall", bufs=1, space="PSUM") as psmall_pool,
        ):
            ident = const_pool.tile([128, 128], FP32)
            make_identity(nc, ident[:, :])

            # ---- routing: alphas = softmax(cond @ alpha_w + alpha_b) [NB, E]
            condT = small_pool.tile([CD, NB], FP32)
            nc.gpsimd.dma_start(
                out=condT[:, :],
                in_=bass.AP(tensor=cond_in, offset=0, ap=[[1, CD], [CD, NB]]),
            )
            aw_sb = small_pool.tile([CD, E], FP32)
            nc.sync.dma_start(out=aw_sb[:, :], in_=aw_in[:, :])
            ab_bc = small_pool.tile([NB, E], FP32)
            nc.gpsimd.dma_start(
                out=ab_bc[:, :],
                in_=bass.AP(tensor=ab_in, offset=0, ap=[[0, NB], [1, E]]),
            )
            p_log = psmall_pool.tile([NB, E], FP32, tag="ps")
            nc.tensor.matmul(p_log[:, :], condT[:, :], aw_sb[:, :], start=True, stop=True)
            logits = small_pool.tile([NB, E], FP32)
            nc.vector.tensor_add(logits[:, :], p_log[:, :], ab_bc[:, :])
            aexp = small_pool.tile([NB, E], FP32)
            nc.scalar.activation(aexp[:, :], logits[:, :], AF.Exp)
            asum = small_pool.tile([NB, 1], FP32)
            nc.vector.reduce_sum(out=asum[:, :], in_=aexp[:, :], axis=mybir.AxisListType.X)
            arec = small_pool.tile([NB, 1], FP32)
            nc.vector.reciprocal(arec[:, :], asum[:, :])
            alphas = small_pool.tile([NB, E], FP32)
            nc.scalar.mul(alphas[:, :], aexp[:, :], arec[:, 0:1])

            # broadcast alphas to all 128 partitions via DRAM round-trip
            adram = dram_pool.tile([NB, E], FP32)
            nc.sync.dma_start(out=adram[:, :], in_=alphas[:, :])
            a_bc = const_pool.tile([128, NB, E], FP32)
            adr_ap = adram[:, :]
            nc.gpsimd.dma_start(
                out=a_bc[:, :, :],
                in_=bass.AP(tensor=adr_ap.tensor, offset=adr_ap.offset,
                            ap=[[0, 128], [E, NB], [1, E]]),
            )

            # ---- mixed bias: bias_fb[f, b] = sum_e expert_bias[e,f] * alphas[b,e]
            aT_ps = psmall_pool.tile([E, NB], FP32, tag="ps")
            nc.tensor.transpose(aT_ps[:, :], alphas[:, :], ident[0:E, 0:NB])
            aT_sb = small_pool.tile([E, NB], FP32)
            nc.vector.tensor_copy(aT_sb[:, :], aT_ps[:, :])
            eb_sb = small_pool.tile([E, F], FP32)
            nc.sync.dma_start(out=eb_sb[:, :], in_=eb_in[:, :])
            pbias = psmall_pool.tile([F, NB], FP32, tag="ps")
            nc.tensor.matmul(pbias[:, :], eb_sb[:, :], aT_sb[:, :], start=True, stop=True)
            bias_fb = const_pool.tile([F, NB], FP32)
            nc.vector.tensor_copy(bias_fb[:, :], pbias[:, :])

            # ---- expert kernel staging (fp32)
            # EK2[p=(half,c), e, kh, f]: half 0 -> kw=0, half 1 -> kw=2
            sE, sKH, sKW, sC = KH * KW * Cin * F, KW * Cin * F, Cin * F, F
            ek2 = ek_pool.tile([128, E, KH, F], FP32)
            nc.sync.dma_start(
                out=ek2[:, :, :, :],
                in_=bass.AP(tensor=ek_in, offset=0,
                            ap=[[sKW, 2], [sC, Cin], [sE, E], [sKH, KH], [1, F]]),
            )
            # EK1[p, e, kh, f]: kw=2 replicated into both partition halves
            ek1 = ek_pool.tile([128, E, KH, F], FP32)
            for half in (0, 1):
                nc.sync.dma_start(
                    out=ek1[half * Cin:(half + 1) * Cin, :, :, :],
                    in_=bass.AP(tensor=ek_in, offset=2 * sKW,
                                ap=[[sC, Cin], [sE, E], [sKH, KH], [1, F]]),
                )

            for b in range(NB):
                # ---- mix weights for sample b (fp32 accumulate, cast to bf16)
                def alpha_ap(e):
                    return a_bc[:, b, e:e + 1]

                def mix(ek_stage, out_tile):
                    acc = mix_pool.tile([128, KH * F], FP32, tag="acc")
                    nc.scalar.mul(
                        acc[:, :],
                        ek_stage[:, 0, :, :].rearrange("p k f -> p (k f)"),
                        alpha_ap(0))
                    for e in range(1, E):
                        src = ek_stage[:, e, :, :].rearrange("p k f -> p (k f)")
                        dst = (acc[:, :] if e < E - 1
                               else out_tile[:, :, :].rearrange("p k f -> p (k f)"))
                        nc.vector.scalar_tensor_tensor(
                            out=dst, in0=src, scalar=alpha_ap(e), in1=acc[:, :],
                            op0=ALU.mult, op1=ALU.add)

                w2b = w_pool.tile([128, KH, F], BF16, tag="w2")
                mix(ek2, w2b)
                w1b = w_pool.tile([128, KH, F], BF16, tag="w1")
                mix(ek1, w1b)

                # ---- load x[b] natural layout: [h, w, c]
                x_h = x_pool.tile([H, W, Cin], FP32)
                for wq in range(4):
                    ws = wq * (W // 4)
                    nc.sync.dma_start(out=x_h[:, ws:ws + W // 4, :],
                                      in_=x_in[b, :, ws:ws + W // 4, :])

                # ---- build S[(half,c), q, hp] bf16; pair q: top=col q-1, bottom=col q
                s_t = s_pool.tile([128, NQ, HP], BF16)
                nc.gpsimd.memset(s_t[:, :, 0:1], 0.0)
                nc.gpsimd.memset(s_t[:, :, HP - 1:HP], 0.0)
                # pair 0 staged (top = col -1 = zeros, bottom = col 0)
                xs0 = mix_pool.tile([H, 2 * Cin], FP32, tag="xs0")
                nc.gpsimd.memset(xs0[:, 0:Cin], 0.0)
                nc.vector.tensor_copy(xs0[:, Cin:2 * Cin], x_h[:, 0, :])
                for k in range(W // 4):
                    ptq = ptin_pool.tile([128, 4, H], FP32, tag="ptin")
                    for j in range(4):
                        q = 4 * k + j
                        if q == 0:
                            nc.tensor.matmul(ptq[:, j, :], xs0[:, :], ident[:, :],
                                             is_transpose=True)
                        else:
                            nc.tensor.matmul(
                                ptq[:, j, :],
                                x_h[:, q - 1:q + 1, :].rearrange("h w c -> h (w c)"),
                                ident[:, :], is_transpose=True)
                    nc.vector.tensor_copy(s_t[:, 4 * k:4 * k + 4, 1:H + 1],
                                          ptq[:, :, :])
                # pair 128: top = col 127, bottom = col 128 (zero pad)
                ptl = ptin_pool.tile([128, 4, H], FP32, tag="ptin")
                nc.tensor.matmul(ptl[0:64, 0, :], x_h[:, W - 1, :], ident[:, :],
                                 is_transpose=True)
                nc.vector.tensor_copy(s_t[0:64, W, 1:H + 1], ptl[0:64, 0, :])
                nc.gpsimd.memset(s_t[64:128, W, :], 0.0)
                # pair 129: cols (128, 129) entirely zero padding
                nc.gpsimd.memset(s_t[:, W + 1, :], 0.0)

                # ---- conv: 32 groups of 4 output columns
                for g in range(W // 4):
                    w0 = 4 * g
                    pc = pconv_pool.tile([F, 4, H], FP32)
                    for dh in range(KH):
                        nc.tensor.matmul(
                            pc[:, :, :], w2b[:, dh, :],
                            s_t[:, w0:w0 + 4, dh:dh + H],
                            start=(dh == 0), stop=False)
                    for dh in range(KH):
                        nc.tensor.matmul(
                            pc[:, :, :], w1b[0:64, dh, :],
                            s_t[0:64, w0 + 2:w0 + 6, dh:dh + H],
                            start=False, stop=(dh == KH - 1))
                    # evacuate + bias (per-partition = per-F)
                    sb1 = sb1_pool.tile([F, 4, H], FP32)
                    nc.scalar.add(sb1[:, :, :], pc[:, :, :], bias_fb[:, b:b + 1])
                    # transpose [F, h] -> [h, F] into one PSUM bank, single evac
                    ptj = ptout_pool.tile([H, 4, F], FP32, tag="ptout")
                    for j in range(4):
                        nc.tensor.matmul(ptj[:, j, :], sb1[:, j, :], ident[:, :],
                                         is_transpose=True)
                    sb2 = sb2_pool.tile([H, 4, F], FP32)
                    nc.scalar.copy(sb2[:, :, :], ptj[:, :, :])
                    nc.sync.dma_start(
                        out=out_t[b, :, w0:w0 + 4, :].rearrange("h w f -> h (w f)"),
                        in_=sb2[:, :, :].rearrange("h w f -> h (w f)"))
    nc.compile()
    return nc


def kernel(x, cond, alpha_w, alpha_b, expert_kernels, expert_bias, trace=False):
    if "nc" not in _cache:
        _cache["nc"] = _build_nc()
    nc = _cache["nc"]
    aw = np.ascontiguousarray(np.asarray(alpha_w, dtype=np.float32))
    ab = np.ascontiguousarray(np.asarray(alpha_b, dtype=np.float32))
    ek = np.ascontiguousarray(np.asarray(expert_kernels, dtype=np.float32))
    eb = np.ascontiguousarray(np.asarray(expert_bias, dtype=np.float32))
    x = np.asarray(x, dtype=np.float32)
    cond = np.asarray(cond, dtype=np.float32)
    in_maps = []
    for c in range(NCORES):
        in_maps.append({
            "x": np.ascontiguousarray(x[c * NB:(c + 1) * NB]),
            "cond": np.ascontiguousarray(cond[c * NB:(c + 1) * NB]),
            "alpha_w": aw, "alpha_b": ab,
            "expert_kernels": ek, "expert_bias": eb,
        })
    res = run_bass_kernel_spmd(nc, in_maps, core_ids=list(range(NCORES)), trace=trace)
    _cache["last_result"] = res
    return np.concatenate([r["out"] for r in res.results], axis=0)



# revision 9
# speedup vs baseline: 1.5770x; 1.5770x over previous
"""CondConv2D Trainium2 kernel: data-parallel over batch across 8 NeuronCores.

Per core (4 samples):
  1. alphas = softmax(cond @ alpha_w + alpha_b)              [tiny PE matmul + ACT/DVE]
  2. K_mix[b] = sum_e alphas[b,e] * expert_kernels[e]        [ACT mul + DVE scalar_tensor_tensor]
  3. conv2d(x[b], K_mix[b], SAME) + bias_mix[b]

Conv strategy (x-stationary, h-major output; non-overlapping column pairs):
  x[b] is SWDGE-cast-loaded as bf16 in natural [h, (w,c)] layout. 64 PE
  transposes of disjoint 2-column blocks build S[(c,2), pk, hp] bf16 where
  pair pk holds cols (2pk-2, 2pk-1); pk=0/65 zero pads, hp pads rows
  (ACT evacuates the transpose PSUM). Conv matmuls put the S-patch as the
  stationary operand and mixed weights as the moving operand, so output
  lands as [h, (w,F)] in PSUM — already HBM-ordered, no output transpose.
  Per group of 4 output columns one PSUM bank [H, 4, F] fp32 accumulates
  24 matmuls (N=F): per column per kh, one full-K pair matmul (even cols
  [W1;W2], odd cols [W0;W1]) + one K=64 edge matmul (W0 on pair bottoms /
  W2 on pair tops). A single DVE tensor_add fuses the broadcast bias and
  the bf16 cast while evacuating into a per-sample [H, W, F] buffer,
  stored to HBM in 4KB-run chunks. Output HBM tensor is bf16; the host
  casts back to fp32. Alphas/bias rows are distributed across partitions
  with gpsimd partition_broadcast (no DRAM round-trip).
"""

import numpy as np

import concourse.bass as bass
import concourse.bacc as bacc
import concourse.mybir as mybir
import concourse.tile as tile
from concourse.bass_utils import run_bass_kernel_spmd
from concourse.masks import make_identity

B, H, W, Cin, E, F = 32, 128, 128, 64, 4, 128
KH = KW = 3
NCORES = 8
NB = B // NCORES  # 4 samples per core
CD = 64  # cond dim
HP = H + 2  # padded row index j; row = j-1
NPK = W // 2 + 2  # 66 pairs; pair pk = cols (2pk-2, 2pk-1); pk 0 and 65 zero

FP32 = mybir.dt.float32
BF16 = mybir.dt.bfloat16
AF = mybir.ActivationFunctionType
ALU = mybir.AluOpType

_cache = {}


def _build_nc():
    nc = bacc.Bacc(None)
    x_in = nc.dram_tensor("x", [NB, H, W, Cin], FP32, kind="ExternalInput")
    cond_in = nc.dram_tensor("cond", [NB, CD], FP32, kind="ExternalInput")
    aw_in = nc.dram_tensor("alpha_w", [CD, E], FP32, kind="ExternalInput")
    ab_in = nc.dram_tensor("alpha_b", [E], FP32, kind="ExternalInput")
    ek_in = nc.dram_tensor("expert_kernels", [E, KH, KW, Cin, F], FP32, kind="ExternalInput")
    eb_in = nc.dram_tensor("expert_bias", [E, F], FP32, kind="ExternalInput")
    out_t = nc.dram_tensor("out", [NB, H, W, F], BF16, kind="ExternalOutput")

    with tile.TileContext(nc) as tc:
        with (
            tc.tile_pool(name="const", bufs=1) as const_pool,
            tc.tile_pool(name="ek", bufs=1) as ek_pool,
            tc.tile_pool(name="mix", bufs=2) as mix_pool,
            tc.tile_pool(name="wts", bufs=2) as w_pool,
            tc.tile_pool(name="xin", bufs=2) as x_pool,
            tc.tile_pool(name="stk", bufs=3) as s_pool,
            tc.tile_pool(name="outb", bufs=2) as out_pool,
            tc.tile_pool(name="small", bufs=2) as small_pool,
            tc.tile_pool(name="dram", bufs=1, space="DRAM") as dram_pool,
            tc.tile_pool(name="pconv", bufs=4, space="PSUM") as pconv_pool,
            tc.tile_pool(name="ptin", bufs=2, space="PSUM") as ptin_pool,
            tc.tile_pool(name="psmall", bufs=1, space="PSUM") as psmall_pool,
        ):
            # identities first: Pool ops gating the PE transposes
            identb = const_pool.tile([128, 128], BF16)
            make_identity(nc, identb[:, :])
            identE = const_pool.tile([NB, NB], FP32)
            make_identity(nc, identE[:, :])

            # alpha_b broadcast [NB, E] (independent, SWDGE stride-0)
            ab_bc = small_pool.tile([NB, E], FP32)
            nc.gpsimd.dma_start(
                out=ab_bc[:, :],
                in_=bass.AP(tensor=ab_in, offset=0, ap=[[0, NB], [1, E]]),
            )

            # first two sample x loads, chunked so transposes can start early
            x_tiles = {}

            def load_x(b, chunks=1):
                xt = x_pool.tile([H, W, Cin], BF16, tag="x", name=f"x_h{b}")
                wq = W // chunks
                for qc in range(chunks):
                    nc.gpsimd.dma_start(
                        out=xt[:, wq * qc:wq * qc + wq, :],
                        in_=x_in[b, :, wq * qc:wq * qc + wq, :])
                x_tiles[b] = xt

            load_x(0, chunks=4)

            # ---- routing: alphas = softmax(cond @ alpha_w + alpha_b) [NB, E]
            condT = small_pool.tile([CD, NB], FP32)
            nc.sync.dma_start(
                out=condT[:, :],
                in_=bass.AP(tensor=cond_in, offset=0, ap=[[1, CD], [CD, NB]]),
            )
            aw_sb = small_pool.tile([CD, E], FP32)
            nc.sync.dma_start(out=aw_sb[:, :], in_=aw_in[:, :])
            p_log = psmall_pool.tile([NB, E], FP32, tag="ps")
            nc.tensor.matmul(p_log[:, :], condT[:, :], aw_sb[:, :], start=True, stop=True)
            logits = small_pool.tile([NB, E], FP32)
            nc.vector.tensor_add(logits[:, :], p_log[:, :], ab_bc[:, :])
            aexp = small_pool.tile([NB, E], FP32)
            nc.scalar.activation(aexp[:, :], logits[:, :], AF.Exp)
            asum = small_pool.tile([NB, 1], FP32)
            nc.vector.reduce_sum(out=asum[:, :], in_=aexp[:, :], axis=mybir.AxisListType.X)
            arec = small_pool.tile([NB, 1], FP32)
            nc.vector.reciprocal(arec[:, :], asum[:, :])
            alphas = small_pool.tile([NB, E], FP32)
            nc.scalar.mul(alphas[:, :], aexp[:, :], arec[:, 0:1])

            # broadcast alphas to all 128 partitions via DRAM round-trip
            adram = dram_pool.tile([NB, E], FP32)
            nc.sync.dma_start(out=adram[:, :], in_=alphas[:, :])
            a_bc = const_pool.tile([128, NB, E], FP32)
            adr_ap = adram[:, :]
            nc.gpsimd.dma_start(
                out=a_bc[:, :, :],
                in_=bass.AP(tensor=adr_ap.tensor, offset=adr_ap.offset,
                            ap=[[0, 128], [E, NB], [1, E]]),
            )

            # ---- mixed bias rows: biasT[b, f] = sum_e alphas[b,e] expert_bias[e,f]
            aT_ps = psmall_pool.tile([E, NB], FP32, tag="ps")
            nc.tensor.transpose(aT_ps[:, :], alphas[:, :], identE[0:E, 0:NB])
            aT_sb = small_pool.tile([E, NB], FP32)
            nc.vector.tensor_copy(aT_sb[:, :], aT_ps[:, :])
            eb_sb = small_pool.tile([E, F], FP32)
            nc.sync.dma_start(out=eb_sb[:, :], in_=eb_in[:, :])
            pbT = psmall_pool.tile([NB, F], FP32, tag="ps")
            nc.tensor.matmul(pbT[:, :], aT_sb[:, :], eb_sb[:, :], start=True, stop=True)
            biasT_sb = small_pool.tile([NB, F], FP32)
            nc.vector.tensor_copy(biasT_sb[:, :], pbT[:, :])
            # bias4[p, b, wl, f] = biasT[b, f] on every partition (DRAM trip)
            bdram = dram_pool.tile([NB, F], FP32)
            nc.sync.dma_start(out=bdram[:, :], in_=biasT_sb[:, :])
            bias4 = const_pool.tile([128, NB, 4, F], FP32)
            bdr_ap = bdram[:, :]
            for wl in range(4):
                nc.gpsimd.dma_start(
                    out=bias4[:, :, wl, :],
                    in_=bass.AP(tensor=bdr_ap.tensor, offset=bdr_ap.offset,
                                ap=[[0, 128], [F, NB], [1, F]]),
                )

            # ---- expert kernel staging (fp32)
            # EA[p=(kw01,c)], EB[p=(kw12,c)], EC[p=(kw2|kw0,c)] each [., e, kh, f]
            sE, sKH, sKW, sC = KH * KW * Cin * F, KW * Cin * F, Cin * F, F
            ek_a = ek_pool.tile([128, E, KH, F], FP32)
            nc.sync.dma_start(
                out=ek_a[:, :, :, :],
                in_=bass.AP(tensor=ek_in, offset=0,
                            ap=[[sKW, 2], [sC, Cin], [sE, E], [sKH, KH], [1, F]]),
            )
            ek_b = ek_pool.tile([128, E, KH, F], FP32)
            nc.sync.dma_start(
                out=ek_b[:, :, :, :],
                in_=bass.AP(tensor=ek_in, offset=sKW,
                            ap=[[sKW, 2], [sC, Cin], [sE, E], [sKH, KH], [1, F]]),
            )
            ek_c = ek_pool.tile([128, E, KH, F], FP32)
            nc.sync.dma_start(
                out=ek_c[0:Cin, :, :, :],
                in_=bass.AP(tensor=ek_in, offset=2 * sKW,
                            ap=[[sC, Cin], [sE, E], [sKH, KH], [1, F]]),
            )
            nc.sync.dma_start(
                out=ek_c[Cin:128, :, :, :],
                in_=bass.AP(tensor=ek_in, offset=0,
                            ap=[[sC, Cin], [sE, E], [sKH, KH], [1, F]]),
            )

            # ---- per-sample weight mixing (fp32 accumulate, cast to bf16)
            mixed = {}

            def issue_mix(b):
                def alpha_ap(e):
                    return a_bc[:, b, e:e + 1]

                def mix(ek_stage, out_tile):
                    acc = mix_pool.tile([128, KH * F], FP32, tag="acc")
                    nc.scalar.mul(
                        acc[:, :],
                        ek_stage[:, 0, :, :].rearrange("p k f -> p (k f)"),
                        alpha_ap(0))
                    for e in range(1, E):
                        src = ek_stage[:, e, :, :].rearrange("p k f -> p (k f)")
                        dst = (acc[:, :] if e < E - 1
                               else out_tile[:, :, :].rearrange("p k f -> p (k f)"))
                        nc.vector.scalar_tensor_tensor(
                            out=dst, in0=src, scalar=alpha_ap(e), in1=acc[:, :],
                            op0=ALU.mult, op1=ALU.add)

                wa = w_pool.tile([128, KH, F], BF16, tag="wa")
                mix(ek_a, wa)
                wb_ = w_pool.tile([128, KH, F], BF16, tag="wb")
                mix(ek_b, wb_)
                wc = w_pool.tile([128, KH, F], BF16, tag="wc")
                mix(ek_c, wc)
                mixed[b] = (wa, wb_, wc)

            s_tiles = {}

            def build_s(b, first=False):
                x_h = x_tiles[b]
                s_t = s_pool.tile([128, NPK, HP], BF16, tag="s", name=f"s_t{b}")
                for kt in range(W // 8):
                    ptq = ptin_pool.tile([128, 4, H], BF16, tag="ptin")
                    for jj in range(4):
                        k = 4 * kt + jj
                        nc.tensor.matmul(
                            ptq[:, jj, :],
                            x_h[:, 2 * k:2 * k + 2, :].rearrange("h w c -> h (w c)"),
                            identb[:, :], is_transpose=True)
                    # ACT evacuates the transpose psum (DVE stays on conv evac)
                    nc.scalar.copy(s_t[:, 4 * kt + 1:4 * kt + 5, 1:H + 1],
                                   ptq[:, :, :])
                    if first and kt == 3:
                        issue_mix(b)
                nc.gpsimd.memset(s_t[:, 0, :], 0.0)
                nc.gpsimd.memset(s_t[:, NPK - 1, :], 0.0)
                nc.gpsimd.memset(s_t[:, :, 0:1], 0.0)
                nc.gpsimd.memset(s_t[:, :, HP - 1:HP], 0.0)
                s_tiles[b] = s_t

            build_s(0, first=True)
            load_x(1, chunks=4)
            build_s(1)

            for b in range(NB):
                wa, wb_, wc = mixed[b]
                s_t = s_tiles[b]
                sb2f = out_pool.tile([H, W, F], BF16, tag="sb2f")
                last = (b == NB - 1)
                for g in range(W // 4):
                    pk = 2 * g
                    pc = pconv_pool.tile([H, 4, F], FP32, tag="pc")
                    nmm = 0
                    for dh in range(KH):
                        for wl in range(4):
                            # col w = 4g + wl; even: full pair [W1;W2] on pair
                            # (k+1); odd: [W0;W1]. k = w//2.
                            wcol = 4 * g + wl
                            k = wcol // 2
                            full_w = wb_ if wcol % 2 == 0 else wa
                            nc.tensor.matmul(
                                pc[:, wl, :], s_t[:, k + 1, dh:dh + H],
                                full_w[:, dh, :],
                                start=(nmm == 0), stop=False)
                            nmm += 1
                            if wcol % 2 == 0:
                                # tap kw=0: col w-1 = bottom of pair k
                                nc.tensor.matmul(
                                    pc[:, wl, :], s_t[64:128, k, dh:dh + H],
                                    wc[64:128, dh, :],
                                    start=False, stop=(nmm == 23))
                            else:
                                # tap kw=2: col w+1 = top of pair k+2
                                nc.tensor.matmul(
                                    pc[:, wl, :], s_t[0:64, k + 2, dh:dh + H],
                                    wc[0:64, dh, :],
                                    start=False, stop=(nmm == 23))
                            nmm += 1
                    if g == 12 and b + 2 < NB:
                        load_x(b + 2)
                    if g == 16 and b + 2 < NB:
                        build_s(b + 2)
                    if g == 24 and b + 1 < NB:
                        issue_mix(b + 1)
                    # evacuate: fused bias add + bf16 cast (DVE)
                    nc.vector.tensor_add(sb2f[:, 4 * g:4 * g + 4, :],
                                         pc[:, :, :], bias4[:, b, :, :])
                    if last and g >= 24:
                        nc.sync.dma_start(
                            out=out_t[b, :, 4 * g:4 * g + 4, :],
                            in_=sb2f[:, 4 * g:4 * g + 4, :])
                    elif g % 4 == 3 and (not last or g < 24):
                        q = g // 4
                        nc.sync.dma_start(
                            out=out_t[b, :, 16 * q:16 * q + 16, :],
                            in_=sb2f[:, 16 * q:16 * q + 16, :])
    nc.compile()
    return nc


def kernel(x, cond, alpha_w, alpha_b, expert_kernels, expert_bias, trace=False):
    if "nc" not in _cache:
        _cache["nc"] = _build_nc()
    nc = _cache["nc"]
    aw = np.ascontiguousarray(np.asarray(alpha_w, dtype=np.float32))
    ab = np.ascontiguousarray(np.asarray(alpha_b, dtype=np.float32))
    ek = np.ascontiguousarray(np.asarray(expert_kernels, dtype=np.float32))
    eb = np.ascontiguousarray(np.asarray(expert_bias, dtype=np.float32))
    x = np.asarray(x, dtype=np.float32)
    cond = np.asarray(cond, dtype=np.float32)
    in_maps = []
    for c in range(NCORES):
        in_maps.append({
            "x": np.ascontiguousarray(x[c * NB:(c + 1) * NB]),
            "cond": np.ascontiguousarray(cond[c * NB:(c + 1) * NB]),
            "alpha_w": aw, "alpha_b": ab,
            "expert_kernels": ek, "expert_bias": eb,
        })
    res = run_bass_kernel_spmd(nc, in_maps, core_ids=list(range(NCORES)), trace=trace)
    _cache["last_result"] = res
    return np.concatenate(
        [np.asarray(r["out"], dtype=np.float32) for r in res.results], axis=0)


# revision 20
# speedup vs baseline: 1.5975x; 1.0130x over previous
"""CondConv2D Trainium2 kernel: data-parallel over batch across 8 NeuronCores.

Per core (4 samples):
  1. alphas = softmax(cond @ alpha_w + alpha_b)   [alpha_b folded into the
     matmul via an appended ones-row, tiny PE matmul + ACT/DVE softmax]
  2. K_mix[b] = sum_e alphas[b,e] * expert_kernels[e]
  3. conv2d(x[b], K_mix[b], SAME) + bias_mix[b]

Conv strategy (x-stationary, h-major output; non-overlapping column pairs):
  x[b] is SWDGE-cast-loaded as bf16 in natural [h, (w,c)] layout (sample 0
  chunked so PE transposes start early). 64 PE transposes of disjoint
  2-column blocks build S[(c,2), pk, hp] bf16 where pair pk holds cols
  (2pk-2, 2pk-1); pk=0/65 zero pads, hp pads rows (ACT evacuates the
  transpose PSUM, DVE memsets the pads). Conv matmuls put the S-patch as
  the stationary operand and mixed weights as the moving operand, so
  output lands as [h, (w,F)] in PSUM — already HBM-ordered, no output
  transpose. Per group of 4 output columns one PSUM bank [H, 4, F] fp32
  accumulates 24 matmuls (N=F): per column per kh, one full-K pair matmul
  (even cols [W1;W2], odd cols [W0;W1]) + one K=64 edge matmul (W0 on pair
  bottoms / W2 on pair tops). A single DVE tensor_add fuses the broadcast
  bias and the bf16 cast while evacuating into a per-sample [H, W, F]
  buffer, stored to HBM in 4KB-run chunks. Output HBM tensor is bf16; the
  host casts back to fp32. Expert kernels stage as bf16 (SWDGE cast DMA).
  Alphas/bias rows reach all partitions without DRAM round-trips: a tiny
  PE matmul spreads them to partitions {0,32,64,96}, then 32-aligned
  gpsimd partition_broadcasts fan out.
"""

import numpy as np

import concourse.bass as bass
import concourse.bacc as bacc
import concourse.mybir as mybir
import concourse.tile as tile
from concourse.bass_utils import run_bass_kernel_spmd
from concourse.masks import make_identity

B, H, W, Cin, E, F = 32, 128, 128, 64, 4, 128
KH = KW = 3
NCORES = 8
NB = B // NCORES  # 4 samples per core
CD = 64  # cond dim
HP = H + 2  # padded row index j; row = j-1
NPK = W // 2 + 2  # 66 pairs; pair pk = cols (2pk-2, 2pk-1); pk 0 and 65 zero

FP32 = mybir.dt.float32
BF16 = mybir.dt.bfloat16
AF = mybir.ActivationFunctionType
ALU = mybir.AluOpType

_cache = {}


def _build_nc():
    nc = bacc.Bacc(None)
    x_in = nc.dram_tensor("x", [NB, H, W, Cin], FP32, kind="ExternalInput")
    cond_in = nc.dram_tensor("cond", [NB, CD], FP32, kind="ExternalInput")
    aw_in = nc.dram_tensor("alpha_w", [CD, E], FP32, kind="ExternalInput")
    ab_in = nc.dram_tensor("alpha_b", [E], FP32, kind="ExternalInput")
    ek_in = nc.dram_tensor("expert_kernels", [E, KH, KW, Cin, F], FP32, kind="ExternalInput")
    eb_in = nc.dram_tensor("expert_bias", [E, F], FP32, kind="ExternalInput")
    out_t = nc.dram_tensor("out", [NB, H, W, F], BF16, kind="ExternalOutput")

    with tile.TileContext(nc) as tc:
        with (
            tc.tile_pool(name="const", bufs=1) as const_pool,
            tc.tile_pool(name="ek", bufs=1) as ek_pool,
            tc.tile_pool(name="mix", bufs=2) as mix_pool,
            tc.tile_pool(name="wts", bufs=2) as w_pool,
            tc.tile_pool(name="xin", bufs=2) as x_pool,
            tc.tile_pool(name="stk", bufs=3) as s_pool,
            tc.tile_pool(name="outb", bufs=2) as out_pool,
            tc.tile_pool(name="small", bufs=2) as small_pool,
            tc.tile_pool(name="dram", bufs=1, space="DRAM") as dram_pool,
            tc.tile_pool(name="pconv", bufs=4, space="PSUM") as pconv_pool,
            tc.tile_pool(name="ptin", bufs=2, space="PSUM") as ptin_pool,
            tc.tile_pool(name="psmall", bufs=1, space="PSUM") as psmall_pool,
        ):
            # small constants + ACT Exp table warmup (DVE/ACT, no DMA)
            warm = const_pool.tile([1, 4], FP32)
            nc.vector.memset(warm[:, :], 0.0)
            nc.scalar.activation(warm[:, :], warm[:, :], AF.Exp)

            # identity first: Pool op gating the PE transposes
            identb = const_pool.tile([128, 128], BF16)
            make_identity(nc, identb[:, :])
            identE = const_pool.tile([NB, NB], FP32)
            make_identity(nc, identE[:, :])

            # routing input loads (tiny, front of the serial DMA queue)
            condT = small_pool.tile([CD + 1, NB], FP32)
            nc.sync.dma_start(
                out=condT[0:CD, :],
                in_=bass.AP(tensor=cond_in, offset=0, ap=[[1, CD], [CD, NB]]),
            )
            nc.vector.memset(condT[CD:CD + 1, :], 1.0)
            aw2 = small_pool.tile([CD + 1, E], FP32)
            nc.sync.dma_start(out=aw2[0:CD, :], in_=aw_in[:, :])
            nc.sync.dma_start(
                out=aw2[CD:CD + 1, :],
                in_=bass.AP(tensor=ab_in, offset=0, ap=[[0, 1], [1, E]]),
            )
            eb_sb = small_pool.tile([E, F], FP32)
            nc.sync.dma_start(out=eb_sb[:, :], in_=eb_in[:, :])

            sE, sKH, sKW, sC = KH * KW * Cin * F, KW * Cin * F, Cin * F, F
            x_tiles = {}

            def load_x(b, chunks=1):
                xt = x_pool.tile([H, W, Cin], BF16, tag="x", name=f"x_h{b}")
                wq = W // chunks
                for qc in range(chunks):
                    nc.gpsimd.dma_start(
                        out=xt[:, wq * qc:wq * qc + wq, :],
                        in_=x_in[b, :, wq * qc:wq * qc + wq, :])
                x_tiles[b] = xt

            # Pool/SWDGE issue order drives the serial DMA schedule
            xt0 = x_pool.tile([H, W, Cin], BF16, tag="x", name="x_h0")
            x_tiles[0] = xt0
            nc.gpsimd.dma_start(out=xt0[:, 0:32, :], in_=x_in[0, :, 0:32, :])

            # expert kernel staging (bf16 via SWDGE cast; 512B source runs)
            ek_a = ek_pool.tile([128, E, KH, F], BF16)
            nc.gpsimd.dma_start(
                out=ek_a[:, :, :, :].rearrange("p e k f -> p (e k f)"),
                in_=bass.AP(tensor=ek_in, offset=0,
                            ap=[[sKW, 2], [sC, Cin], [sE, E], [sKH, KH], [1, F]]),
            )
            nc.gpsimd.dma_start(out=xt0[:, 32:64, :], in_=x_in[0, :, 32:64, :])

            # ---- routing: alphas = softmax([cond, 1] @ [alpha_w; alpha_b])
            p_log = psmall_pool.tile([NB, E], FP32, tag="ps")
            nc.tensor.matmul(p_log[:, :], condT[:, :], aw2[:, :], start=True, stop=True)
            aexp = small_pool.tile([NB, E], FP32)
            nc.scalar.activation(aexp[:, :], p_log[:, :], AF.Exp)
            asum = small_pool.tile([NB, 1], FP32)
            nc.vector.reduce_sum(out=asum[:, :], in_=aexp[:, :], axis=mybir.AxisListType.X)
            arec = small_pool.tile([NB, 1], FP32)
            nc.vector.reciprocal(arec[:, :], asum[:, :])
            alphas = small_pool.tile([NB, E], FP32)
            nc.scalar.mul(alphas[:, :], aexp[:, :], arec[:, 0:1])

            # broadcast alphas to all 128 partitions via DRAM round-trip
            adram = dram_pool.tile([NB, E], FP32)
            nc.sync.dma_start(out=adram[:, :], in_=alphas[:, :])
            a_bc = const_pool.tile([128, NB, E], FP32)
            adr_ap = adram[:, :]
            nc.gpsimd.dma_start(
                out=a_bc[:, :, :],
                in_=bass.AP(tensor=adr_ap.tensor, offset=adr_ap.offset,
                            ap=[[0, 128], [E, NB], [1, E]]),
            )

            ek_b = ek_pool.tile([128, E, KH, F], BF16)
            nc.gpsimd.dma_start(
                out=ek_b[:, :, :, :].rearrange("p e k f -> p (e k f)"),
                in_=bass.AP(tensor=ek_in, offset=sKW,
                            ap=[[sKW, 2], [sC, Cin], [sE, E], [sKH, KH], [1, F]]),
            )
            ek_c = ek_pool.tile([128, E, KH, F], BF16)
            nc.gpsimd.dma_start(
                out=ek_c[0:Cin, :, :, :].rearrange("p e k f -> p (e k f)"),
                in_=bass.AP(tensor=ek_in, offset=2 * sKW,
                            ap=[[sC, Cin], [sE, E], [sKH, KH], [1, F]]),
            )
            nc.gpsimd.dma_start(
                out=ek_c[Cin:128, :, :, :].rearrange("p e k f -> p (e k f)"),
                in_=bass.AP(tensor=ek_in, offset=0,
                            ap=[[sC, Cin], [sE, E], [sKH, KH], [1, F]]),
            )
            nc.gpsimd.dma_start(out=xt0[:, 64:128, :], in_=x_in[0, :, 64:128, :])

            # ---- mixed bias rows: biasT[b, f] = sum_e alphas[b,e] expert_bias[e,f]
            aT_ps = psmall_pool.tile([E, NB], FP32, tag="ps")
            nc.tensor.transpose(aT_ps[:, :], alphas[:, :], identE[0:E, 0:NB])
            aT_sb = small_pool.tile([E, NB], FP32)
            nc.vector.tensor_copy(aT_sb[:, :], aT_ps[:, :])
            pbT = psmall_pool.tile([NB, F], FP32, tag="ps")
            nc.tensor.matmul(pbT[:, :], aT_sb[:, :], eb_sb[:, :], start=True, stop=True)
            biasT_sb = small_pool.tile([NB, F], FP32)
            nc.vector.tensor_copy(biasT_sb[:, :], pbT[:, :])
            # bias4[p, b, wl, f] = biasT[b, f] on every partition (DRAM trip)
            bdram = dram_pool.tile([NB, F], FP32)
            nc.sync.dma_start(out=bdram[:, :], in_=biasT_sb[:, :])
            bias4 = const_pool.tile([128, NB, 4, F], FP32)
            bdr_ap = bdram[:, :]
            for wl in range(4):
                nc.gpsimd.dma_start(
                    out=bias4[:, :, wl, :],
                    in_=bass.AP(tensor=bdr_ap.tensor, offset=bdr_ap.offset,
                                ap=[[0, 128], [F, NB], [1, F]]),
                )

            # ---- per-sample weight mixing (bf16 accumulate, all DVE)
            mixed = {}

            def issue_mix(b):
                def alpha_ap(e):
                    return a_bc[:, b, e:e + 1]

                def mix(ek_stage, out_tile):
                    acc = mix_pool.tile([128, KH * F], BF16, tag="acc")
                    nc.vector.tensor_scalar_mul(
                        acc[:, :],
                        ek_stage[:, 0, :, :].rearrange("p k f -> p (k f)"),
                        alpha_ap(0))
                    for e in range(1, E):
                        src = ek_stage[:, e, :, :].rearrange("p k f -> p (k f)")
                        dst = (acc[:, :] if e < E - 1
                               else out_tile[:, :, :].rearrange("p k f -> p (k f)"))
                        nc.vector.scalar_tensor_tensor(
                            out=dst, in0=src, scalar=alpha_ap(e), in1=acc[:, :],
                            op0=ALU.mult, op1=ALU.add)

                wa = w_pool.tile([128, KH, F], BF16, tag="wa")
                mix(ek_a, wa)
                wb_ = w_pool.tile([128, KH, F], BF16, tag="wb")
                mix(ek_b, wb_)
                wc = w_pool.tile([128, KH, F], BF16, tag="wc")
                mix(ek_c, wc)
                mixed[b] = (wa, wb_, wc)

            s_tiles = {}

            def build_s(b, kt_lo=0, kt_hi=W // 8, first=False):
                x_h = x_tiles[b]
                if kt_lo == 0:
                    s_t = s_pool.tile([128, NPK, HP], BF16, tag="s", name=f"s_t{b}")
                    nc.vector.memset(s_t[:, 0, :], 0.0)
                    nc.vector.memset(s_t[:, NPK - 1, :], 0.0)
                    nc.vector.memset(s_t[:, :, 0:1], 0.0)
                    nc.vector.memset(s_t[:, :, HP - 1:HP], 0.0)
                    s_tiles[b] = s_t
                s_t = s_tiles[b]
                for kt in range(kt_lo, kt_hi):
                    ptq = ptin_pool.tile([128, 4, H], BF16, tag="ptin")
                    for jj in range(4):
                        k = 4 * kt + jj
                        nc.tensor.matmul(
                            ptq[:, jj, :],
                            x_h[:, 2 * k:2 * k + 2, :].rearrange("h w c -> h (w c)"),
                            identb[:, :], is_transpose=True)
                    # ACT evacuates the transpose psum (DVE stays on conv evac)
                    nc.scalar.copy(s_t[:, 4 * kt + 1:4 * kt + 5, 1:H + 1],
                                   ptq[:, :, :])
                    if first and kt == 1:
                        issue_mix(b)

            build_s(0, 0, 8, first=True)

            for b in range(NB):
                wa, wb_, wc = mixed[b]
                s_t = s_tiles[b]
                sb2f = out_pool.tile([H, W, F], BF16, tag="sb2f")
                last = (b == NB - 1)
                for g in range(W // 4):
                    pk = 2 * g
                    pc = pconv_pool.tile([H, 4, F], FP32, tag="pc")
                    nmm = 0
                    for dh in range(KH):
                        for wl in range(4):
                            # col w = 4g + wl; even: full pair [W1;W2] on pair
                            # (k+1); odd: [W0;W1]. k = w//2.
                            wcol = 4 * g + wl
                            k = wcol // 2
                            full_w = wb_ if wcol % 2 == 0 else wa
                            nc.tensor.matmul(
                                pc[:, wl, :], s_t[:, k + 1, dh:dh + H],
                                full_w[:, dh, :],
                                start=(nmm == 0), stop=False)
                            nmm += 1
                            if wcol % 2 == 0:
                                # tap kw=0: col w-1 = bottom of pair k
                                nc.tensor.matmul(
                                    pc[:, wl, :], s_t[64:128, k, dh:dh + H],
                                    wc[64:128, dh, :],
                                    start=False, stop=(nmm == 23))
                            else:
                                # tap kw=2: col w+1 = top of pair k+2
                                nc.tensor.matmul(
                                    pc[:, wl, :], s_t[0:64, k + 2, dh:dh + H],
                                    wc[0:64, dh, :],
                                    start=False, stop=(nmm == 23))
                            nmm += 1
                    if g == 2 and b == 0:
                        build_s(0, 8, 16)
                    if g == 0 and b + 1 < NB:
                        load_x(b + 1)
                    if g == 16 and b + 1 < NB:
                        build_s(b + 1)
                    if g == 24 and b + 1 < NB:
                        issue_mix(b + 1)
                    # evacuate: fused bias add + bf16 cast (DVE)
                    nc.vector.tensor_add(sb2f[:, 4 * g:4 * g + 4, :],
                                         pc[:, :, :], bias4[:, b, :, :])
                    if last and g >= 24:
                        nc.sync.dma_start(
                            out=out_t[b, :, 4 * g:4 * g + 4, :],
                            in_=sb2f[:, 4 * g:4 * g + 4, :])
                    elif g % 4 == 3 and (not last or g < 24):
                        q = g // 4
                        nc.sync.dma_start(
                            out=out_t[b, :, 16 * q:16 * q + 16, :],
                            in_=sb2f[:, 16 * q:16 * q + 16, :])
    nc.compile()
    return nc


def kernel(x, cond, alpha_w, alpha_b, expert_kernels, expert_bias, trace=False):
    if "nc" not in _cache:
        _cache["nc"] = _build_nc()
    nc = _cache["nc"]
    aw = np.ascontiguousarray(np.asarray(alpha_w, dtype=np.float32))
    ab = np.ascontiguousarray(np.asarray(alpha_b, dtype=np.float32))
    ek = np.ascontiguousarray(np.asarray(expert_kernels, dtype=np.float32))
    eb = np.ascontiguousarray(np.asarray(expert_bias, dtype=np.float32))
    x = np.asarray(x, dtype=np.float32)
    cond = np.asarray(cond, dtype=np.float32)
    in_maps = []
    for c in range(NCORES):
        in_maps.append({
            "x": np.ascontiguousarray(x[c * NB:(c + 1) * NB]),
            "cond": np.ascontiguousarray(cond[c * NB:(c + 1) * NB]),
            "alpha_w": aw, "alpha_b": ab,
            "expert_kernels": ek, "expert_bias": eb,
        })
    res = run_bass_kernel_spmd(nc, in_maps, core_ids=list(range(NCORES)), trace=trace)
    _cache["last_result"] = res
    return np.concatenate(
        [np.asarray(r["out"], dtype=np.float32) for r in res.results], axis=0)


# revision 25
# speedup vs baseline: 1.6090x; 1.0072x over previous
"""CondConv2D Trainium2 kernel: data-parallel over batch across 8 NeuronCores.

Per core (4 samples):
  1. alphas = softmax(cond @ alpha_w + alpha_b)   [alpha_b folded into the
     matmul via an appended ones-row, tiny PE matmul + ACT/DVE softmax]
  2. K_mix[b] = sum_e alphas[b,e] * expert_kernels[e]
  3. conv2d(x[b], K_mix[b], SAME) + bias_mix[b]

Conv strategy (x-stationary, h-major output; non-overlapping column pairs):
  x[b] is SWDGE-cast-loaded as bf16 in natural [h, (w,c)] layout (sample 0
  in 4 w-chunks so PE transposes start early). 64 PE transposes of
  disjoint 2-column blocks build S[(c,2), pk, hp] bf16 where pair pk holds
  cols (2pk-2, 2pk-1); pk=0/65 zero pads, hp pads rows (ACT evacuates the
  transpose PSUM, DVE memsets the pads). Conv matmuls put the S-patch as
  the stationary operand and mixed weights as the moving operand, so
  output lands as [h, (w,F)] in PSUM — already HBM-ordered, no output
  transpose. Per group of 4 output columns one PSUM bank [H, 4, F] fp32
  accumulates 24 matmuls (N=F): per column per kh, one full-K pair matmul
  (even cols [W1;W2], odd cols [W0;W1]) + one K=64 edge matmul (W0 on pair
  bottoms / W2 on pair tops). A single DVE tensor_add fuses the broadcast
  bias and the bf16 cast while evacuating into a per-sample [H, W, F]
  buffer, stored to HBM in 4KB-run chunks. Output HBM tensor is bf16; the
  host casts back to fp32. Expert kernels stage as bf16 (SWDGE cast DMA).
"""

import numpy as np

import concourse.bass as bass
import concourse.bacc as bacc
import concourse.mybir as mybir
import concourse.tile as tile
from concourse.bass_utils import run_bass_kernel_spmd
from concourse.masks import make_identity

B, H, W, Cin, E, F = 32, 128, 128, 64, 4, 128
KH = KW = 3
NCORES = 8
NB = B // NCORES  # 4 samples per core
CD = 64  # cond dim
HP = H + 2  # padded row index j; row = j-1
NPK = W // 2 + 2  # 66 pairs; pair pk = cols (2pk-2, 2pk-1); pk 0 and 65 zero

FP32 = mybir.dt.float32
BF16 = mybir.dt.bfloat16
AF = mybir.ActivationFunctionType
ALU = mybir.AluOpType

_cache = {}


def _build_nc():
    nc = bacc.Bacc(None)
    x_in = nc.dram_tensor("x", [NB, H, W, Cin], FP32, kind="ExternalInput")
    cond_in = nc.dram_tensor("cond", [NB, CD], FP32, kind="ExternalInput")
    aw_in = nc.dram_tensor("alpha_w", [CD, E], FP32, kind="ExternalInput")
    ab_in = nc.dram_tensor("alpha_b", [E], FP32, kind="ExternalInput")
    ek_in = nc.dram_tensor("expert_kernels", [E, KH, KW, Cin, F], FP32, kind="ExternalInput")
    eb_in = nc.dram_tensor("expert_bias", [E, F], FP32, kind="ExternalInput")
    out_t = nc.dram_tensor("out", [NB, H, W, F], BF16, kind="ExternalOutput")

    with tile.TileContext(nc) as tc:
        with (
            tc.tile_pool(name="const", bufs=1) as const_pool,
            tc.tile_pool(name="ek", bufs=1) as ek_pool,
            tc.tile_pool(name="mix", bufs=2) as mix_pool,
            tc.tile_pool(name="wts", bufs=2) as w_pool,
            tc.tile_pool(name="xin", bufs=2) as x_pool,
            tc.tile_pool(name="stk", bufs=3) as s_pool,
            tc.tile_pool(name="outb", bufs=2) as out_pool,
            tc.tile_pool(name="small", bufs=2) as small_pool,
            tc.tile_pool(name="dram", bufs=1, space="DRAM") as dram_pool,
            tc.tile_pool(name="pconv", bufs=4, space="PSUM") as pconv_pool,
            tc.tile_pool(name="ptin", bufs=2, space="PSUM") as ptin_pool,
            tc.tile_pool(name="psmall", bufs=1, space="PSUM") as psmall_pool,
        ):
            # identity first: Pool op gating the PE transposes
            identb = const_pool.tile([128, 128], BF16)
            make_identity(nc, identb[:, :])
            identE = const_pool.tile([NB, NB], FP32)
            make_identity(nc, identE[:, :])

            sE, sKH, sKW, sC = KH * KW * Cin * F, KW * Cin * F, Cin * F, F
            x_tiles = {}

            def load_x(b, chunks=1):
                xt = x_pool.tile([H, W, Cin], BF16, tag="x", name=f"x_h{b}")
                wq = W // chunks
                for qc in range(chunks):
                    nc.gpsimd.dma_start(
                        out=xt[:, wq * qc:wq * qc + wq, :],
                        in_=x_in[b, :, wq * qc:wq * qc + wq, :])
                x_tiles[b] = xt

            # Pool/SWDGE issue order drives the serial DMA schedule:
            # x0c0, ek_a, ek_b, ek_c, x0c1, a_bc, bias4, x0c2, x0c3, x1
            xt0 = x_pool.tile([H, W, Cin], BF16, tag="x", name="x_h0")
            x_tiles[0] = xt0
            nc.gpsimd.dma_start(out=xt0[:, 0:32, :], in_=x_in[0, :, 0:32, :])
            nc.gpsimd.dma_start(out=xt0[:, 32:64, :], in_=x_in[0, :, 32:64, :])

            # expert kernel staging (bf16 via SWDGE cast; 512B source runs)
            ek_a = ek_pool.tile([128, E, KH, F], BF16)
            nc.gpsimd.dma_start(
                out=ek_a[:, :, :, :],
                in_=bass.AP(tensor=ek_in, offset=0,
                            ap=[[sKW, 2], [sC, Cin], [sE, E], [sKH, KH], [1, F]]),
            )
            # ---- routing: alphas = softmax([cond, 1] @ [alpha_w; alpha_b])
            condT = small_pool.tile([CD + 1, NB], FP32)
            nc.sync.dma_start(
                out=condT[0:CD, :],
                in_=bass.AP(tensor=cond_in, offset=0, ap=[[1, CD], [CD, NB]]),
            )
            nc.vector.memset(condT[CD:CD + 1, :], 1.0)
            aw2 = small_pool.tile([CD + 1, E], FP32)
            nc.sync.dma_start(out=aw2[0:CD, :], in_=aw_in[:, :])
            nc.sync.dma_start(
                out=aw2[CD:CD + 1, :],
                in_=bass.AP(tensor=ab_in, offset=0, ap=[[0, 1], [1, E]]),
            )
            p_log = psmall_pool.tile([NB, E], FP32, tag="ps")
            nc.tensor.matmul(p_log[:, :], condT[:, :], aw2[:, :], start=True, stop=True)
            aexp = small_pool.tile([NB, E], FP32)
            nc.scalar.activation(aexp[:, :], p_log[:, :], AF.Exp)
            asum = small_pool.tile([NB, 1], FP32)
            nc.vector.reduce_sum(out=asum[:, :], in_=aexp[:, :], axis=mybir.AxisListType.X)
            arec = small_pool.tile([NB, 1], FP32)
            nc.vector.reciprocal(arec[:, :], asum[:, :])
            alphas = small_pool.tile([NB, E], FP32)
            nc.scalar.mul(alphas[:, :], aexp[:, :], arec[:, 0:1])

            # broadcast alphas to all 128 partitions via DRAM round-trip
            adram = dram_pool.tile([NB, E], FP32)
            nc.sync.dma_start(out=adram[:, :], in_=alphas[:, :])
            a_bc = const_pool.tile([128, NB, E], FP32)
            adr_ap = adram[:, :]
            nc.gpsimd.dma_start(
                out=a_bc[:, :, :],
                in_=bass.AP(tensor=adr_ap.tensor, offset=adr_ap.offset,
                            ap=[[0, 128], [E, NB], [1, E]]),
            )

            ek_b = ek_pool.tile([128, E, KH, F], BF16)
            nc.gpsimd.dma_start(
                out=ek_b[:, :, :, :],
                in_=bass.AP(tensor=ek_in, offset=sKW,
                            ap=[[sKW, 2], [sC, Cin], [sE, E], [sKH, KH], [1, F]]),
            )
            ek_c = ek_pool.tile([128, E, KH, F], BF16)
            nc.gpsimd.dma_start(
                out=ek_c[0:Cin, :, :, :],
                in_=bass.AP(tensor=ek_in, offset=2 * sKW,
                            ap=[[sC, Cin], [sE, E], [sKH, KH], [1, F]]),
            )
            nc.gpsimd.dma_start(
                out=ek_c[Cin:128, :, :, :],
                in_=bass.AP(tensor=ek_in, offset=0,
                            ap=[[sC, Cin], [sE, E], [sKH, KH], [1, F]]),
            )
            nc.gpsimd.dma_start(out=xt0[:, 64:96, :], in_=x_in[0, :, 64:96, :])
            nc.gpsimd.dma_start(out=xt0[:, 96:128, :], in_=x_in[0, :, 96:128, :])

            # ---- mixed bias rows: biasT[b, f] = sum_e alphas[b,e] expert_bias[e,f]
            aT_ps = psmall_pool.tile([E, NB], FP32, tag="ps")
            nc.tensor.transpose(aT_ps[:, :], alphas[:, :], identE[0:E, 0:NB])
            aT_sb = small_pool.tile([E, NB], FP32)
            nc.vector.tensor_copy(aT_sb[:, :], aT_ps[:, :])
            eb_sb = small_pool.tile([E, F], FP32)
            nc.sync.dma_start(out=eb_sb[:, :], in_=eb_in[:, :])
            pbT = psmall_pool.tile([NB, F], FP32, tag="ps")
            nc.tensor.matmul(pbT[:, :], aT_sb[:, :], eb_sb[:, :], start=True, stop=True)
            biasT_sb = small_pool.tile([NB, F], FP32)
            nc.vector.tensor_copy(biasT_sb[:, :], pbT[:, :])
            # bias4[p, b, wl, f] = biasT[b, f] on every partition (DRAM trip)
            bdram = dram_pool.tile([NB, F], FP32)
            nc.sync.dma_start(out=bdram[:, :], in_=biasT_sb[:, :])
            bias4 = const_pool.tile([128, NB, 4, F], FP32)
            bdr_ap = bdram[:, :]
            for wl in range(4):
                nc.gpsimd.dma_start(
                    out=bias4[:, :, wl, :],
                    in_=bass.AP(tensor=bdr_ap.tensor, offset=bdr_ap.offset,
                                ap=[[0, 128], [F, NB], [1, F]]),
                )

            # ---- per-sample weight mixing (fp32 accumulate, cast to bf16)
            mixed = {}

            def issue_mix(b):
                def alpha_ap(e):
                    return a_bc[:, b, e:e + 1]

                def mix(ek_stage, out_tile):
                    acc = mix_pool.tile([128, KH * F], FP32, tag="acc")
                    nc.scalar.mul(
                        acc[:, :],
                        ek_stage[:, 0, :, :].rearrange("p k f -> p (k f)"),
                        alpha_ap(0))
                    for e in range(1, E):
                        src = ek_stage[:, e, :, :].rearrange("p k f -> p (k f)")
                        dst = (acc[:, :] if e < E - 1
                               else out_tile[:, :, :].rearrange("p k f -> p (k f)"))
                        nc.vector.scalar_tensor_tensor(
                            out=dst, in0=src, scalar=alpha_ap(e), in1=acc[:, :],
                            op0=ALU.mult, op1=ALU.add)

                wa = w_pool.tile([128, KH, F], BF16, tag="wa")
                mix(ek_a, wa)
                wb_ = w_pool.tile([128, KH, F], BF16, tag="wb")
                mix(ek_b, wb_)
                wc = w_pool.tile([128, KH, F], BF16, tag="wc")
                mix(ek_c, wc)
                mixed[b] = (wa, wb_, wc)

            s_tiles = {}

            def build_s(b, kt_lo=0, kt_hi=W // 8, first=False):
                x_h = x_tiles[b]
                if kt_lo == 0:
                    s_t = s_pool.tile([128, NPK, HP], BF16, tag="s", name=f"s_t{b}")
                    nc.vector.memset(s_t[:, 0, :], 0.0)
                    nc.vector.memset(s_t[:, NPK - 1, :], 0.0)
                    nc.vector.memset(s_t[:, :, 0:1], 0.0)
                    nc.vector.memset(s_t[:, :, HP - 1:HP], 0.0)
                    s_tiles[b] = s_t
                s_t = s_tiles[b]
                for kt in range(kt_lo, kt_hi):
                    ptq = ptin_pool.tile([128, 4, H], BF16, tag="ptin")
                    for jj in range(4):
                        k = 4 * kt + jj
                        nc.tensor.matmul(
                            ptq[:, jj, :],
                            x_h[:, 2 * k:2 * k + 2, :].rearrange("h w c -> h (w c)"),
                            identb[:, :], is_transpose=True)
                    # ACT evacuates the transpose psum (DVE stays on conv evac)
                    nc.scalar.copy(s_t[:, 4 * kt + 1:4 * kt + 5, 1:H + 1],
                                   ptq[:, :, :])
                    if first and kt == 1:
                        issue_mix(b)

            build_s(0, 0, 8, first=True)

            for b in range(NB):
                wa, wb_, wc = mixed[b]
                s_t = s_tiles[b]
                sb2f = out_pool.tile([H, W, F], BF16, tag="sb2f")
                last = (b == NB - 1)
                for g in range(W // 4):
                    pk = 2 * g
                    pc = pconv_pool.tile([H, 4, F], FP32, tag="pc")
                    nmm = 0
                    for dh in range(KH):
                        for wl in range(4):
                            # col w = 4g + wl; even: full pair [W1;W2] on pair
                            # (k+1); odd: [W0;W1]. k = w//2.
                            wcol = 4 * g + wl
                            k = wcol // 2
                            full_w = wb_ if wcol % 2 == 0 else wa
                            nc.tensor.matmul(
                                pc[:, wl, :], s_t[:, k + 1, dh:dh + H],
                                full_w[:, dh, :],
                                start=(nmm == 0), stop=False)
                            nmm += 1
                            if wcol % 2 == 0:
                                # tap kw=0: col w-1 = bottom of pair k
                                nc.tensor.matmul(
                                    pc[:, wl, :], s_t[64:128, k, dh:dh + H],
                                    wc[64:128, dh, :],
                                    start=False, stop=(nmm == 23))
                            else:
                                # tap kw=2: col w+1 = top of pair k+2
                                nc.tensor.matmul(
                                    pc[:, wl, :], s_t[0:64, k + 2, dh:dh + H],
                                    wc[0:64, dh, :],
                                    start=False, stop=(nmm == 23))
                            nmm += 1
                    if g == 2 and b == 0:
                        build_s(0, 8, 16)
                    if g == 0 and b + 1 < NB:
                        load_x(b + 1)
                    if g == 16 and b + 1 < NB:
                        build_s(b + 1)
                    if g == 24 and b + 1 < NB:
                        issue_mix(b + 1)
                    # evacuate: fused bias add + bf16 cast (DVE)
                    nc.vector.tensor_add(sb2f[:, 4 * g:4 * g + 4, :],
                                         pc[:, :, :], bias4[:, b, :, :])
                    if last and g >= 24:
                        nc.sync.dma_start(
                            out=out_t[b, :, 4 * g:4 * g + 4, :],
                            in_=sb2f[:, 4 * g:4 * g + 4, :])
                    elif g % 4 == 3 and (not last or g < 24):
                        q = g // 4
                        nc.sync.dma_start(
                            out=out_t[b, :, 16 * q:16 * q + 16, :],
                            in_=sb2f[:, 16 * q:16 * q + 16, :])
    nc.compile()
    return nc


def kernel(x, cond, alpha_w, alpha_b, expert_kernels, expert_bias, trace=False):
    if "nc" not in _cache:
        _cache["nc"] = _build_nc()
    nc = _cache["nc"]
    aw = np.ascontiguousarray(np.asarray(alpha_w, dtype=np.float32))
    ab = np.ascontiguousarray(np.asarray(alpha_b, dtype=np.float32))
    ek = np.ascontiguousarray(np.asarray(expert_kernels, dtype=np.float32))
    eb = np.ascontiguousarray(np.asarray(expert_bias, dtype=np.float32))
    x = np.asarray(x, dtype=np.float32)
    cond = np.asarray(cond, dtype=np.float32)
    in_maps = []
    for c in range(NCORES):
        in_maps.append({
            "x": np.ascontiguousarray(x[c * NB:(c + 1) * NB]),
            "cond": np.ascontiguousarray(cond[c * NB:(c + 1) * NB]),
            "alpha_w": aw, "alpha_b": ab,
            "expert_kernels": ek, "expert_bias": eb,
        })
    res = run_bass_kernel_spmd(nc, in_maps, core_ids=list(range(NCORES)), trace=trace)
    _cache["last_result"] = res
    return np.concatenate(
        [np.asarray(r["out"], dtype=np.float32) for r in res.results], axis=0)


# revision 30
# speedup vs baseline: 1.6209x; 1.0074x over previous
"""CondConv2D Trainium2 kernel: data-parallel over batch across 8 NeuronCores.

Per core (4 samples):
  1. alphas = softmax(cond @ alpha_w + alpha_b)   [alpha_b folded into the
     matmul via an appended ones-row, tiny PE matmul + ACT/DVE softmax]
  2. K_mix[b] = sum_e alphas[b,e] * expert_kernels[e]
  3. conv2d(x[b], K_mix[b], SAME) + bias_mix[b]

Conv strategy (x-stationary, h-major output; non-overlapping column pairs):
  x[b] is SWDGE-cast-loaded as bf16 in natural [h, (w,c)] layout (sample 0
  in 4 w-chunks so PE transposes start early). 64 PE transposes of
  disjoint 2-column blocks build S[(c,2), pk, hp] bf16 where pair pk holds
  cols (2pk-2, 2pk-1); pk=0/65 zero pads, hp pads rows (ACT evacuates the
  transpose PSUM, DVE memsets the pads). Conv matmuls put the S-patch as
  the stationary operand and mixed weights as the moving operand, so
  output lands as [h, (w,F)] in PSUM — already HBM-ordered, no output
  transpose. Per group of 4 output columns one PSUM bank [H, 4, F] fp32
  accumulates 24 matmuls (N=F): per column per kh, one full-K pair matmul
  (even cols [W1;W2], odd cols [W0;W1]) + one K=64 edge matmul (W0 on pair
  bottoms / W2 on pair tops). A single DVE tensor_add fuses the broadcast
  bias and the bf16 cast while evacuating into a per-sample [H, W, F]
  buffer, stored to HBM in 4KB-run chunks. Output HBM tensor is bf16; the
  host casts back to fp32. Expert kernels stage as bf16 (SWDGE cast DMA).
"""

import numpy as np

import concourse.bass as bass
import concourse.bacc as bacc
import concourse.mybir as mybir
import concourse.tile as tile
from concourse.bass_utils import run_bass_kernel_spmd
from concourse.masks import make_identity

B, H, W, Cin, E, F = 32, 128, 128, 64, 4, 128
KH = KW = 3
NCORES = 8
NB = B // NCORES  # 4 samples per core
CD = 64  # cond dim
HP = H + 2  # padded row index j; row = j-1
NPK = W // 2 + 2  # 66 pairs; pair pk = cols (2pk-2, 2pk-1); pk 0 and 65 zero

FP32 = mybir.dt.float32
BF16 = mybir.dt.bfloat16
AF = mybir.ActivationFunctionType
ALU = mybir.AluOpType

_cache = {}


def _build_nc():
    nc = bacc.Bacc(None)
    x_in = nc.dram_tensor("x", [NB, H, W, Cin], FP32, kind="ExternalInput")
    cond_in = nc.dram_tensor("cond", [NB, CD], FP32, kind="ExternalInput")
    aw_in = nc.dram_tensor("alpha_w", [CD, E], FP32, kind="ExternalInput")
    ab_in = nc.dram_tensor("alpha_b", [E], FP32, kind="ExternalInput")
    ek_in = nc.dram_tensor("expert_kernels", [E, KH, KW, Cin, F], FP32, kind="ExternalInput")
    eb_in = nc.dram_tensor("expert_bias", [E, F], FP32, kind="ExternalInput")
    out_t = nc.dram_tensor("out", [NB, H, W, F], BF16, kind="ExternalOutput")

    with tile.TileContext(nc) as tc:
        with (
            tc.tile_pool(name="const", bufs=1) as const_pool,
            tc.tile_pool(name="ek", bufs=1) as ek_pool,
            tc.tile_pool(name="mix", bufs=2) as mix_pool,
            tc.tile_pool(name="wts", bufs=2) as w_pool,
            tc.tile_pool(name="xin", bufs=2) as x_pool,
            tc.tile_pool(name="stk", bufs=3) as s_pool,
            tc.tile_pool(name="outb", bufs=2) as out_pool,
            tc.tile_pool(name="small", bufs=2) as small_pool,
            tc.tile_pool(name="dram", bufs=1, space="DRAM") as dram_pool,
            tc.tile_pool(name="pconv", bufs=4, space="PSUM") as pconv_pool,
            tc.tile_pool(name="ptin", bufs=2, space="PSUM") as ptin_pool,
            tc.tile_pool(name="psmall", bufs=1, space="PSUM") as psmall_pool,
        ):
            # identity first: Pool op gating the PE transposes
            identb = const_pool.tile([128, 128], BF16)
            make_identity(nc, identb[:, :])
            identE = const_pool.tile([NB, NB], FP32)
            make_identity(nc, identE[:, :])

            sE, sKH, sKW, sC = KH * KW * Cin * F, KW * Cin * F, Cin * F, F
            x_tiles = {}

            def load_x(b, chunks=1):
                xt = x_pool.tile([H, W, Cin], BF16, tag="x", name=f"x_h{b}")
                wq = W // chunks
                for qc in range(chunks):
                    nc.gpsimd.dma_start(
                        out=xt[:, wq * qc:wq * qc + wq, :],
                        in_=x_in[b, :, wq * qc:wq * qc + wq, :])
                x_tiles[b] = xt

            # Pool/SWDGE issue order drives the serial DMA schedule:
            # x0c0, ek_a, ek_b, ek_c, x0c1, a_bc, bias4, x0c2, x0c3, x1
            xt0 = x_pool.tile([H, W, Cin], BF16, tag="x", name="x_h0")
            x_tiles[0] = xt0
            nc.gpsimd.dma_start(out=xt0[:, 0:32, :], in_=x_in[0, :, 0:32, :])
            nc.gpsimd.dma_start(out=xt0[:, 32:64, :], in_=x_in[0, :, 32:64, :])

            # expert kernel staging (bf16 via SWDGE cast; 512B source runs)
            ek_a = ek_pool.tile([128, E, KH, F], BF16)
            nc.gpsimd.dma_start(
                out=ek_a[:, :, :, :],
                in_=bass.AP(tensor=ek_in, offset=0,
                            ap=[[sKW, 2], [sC, Cin], [sE, E], [sKH, KH], [1, F]]),
            )
            # ---- routing: alphas = softmax([cond, 1] @ [alpha_w; alpha_b])
            condT = small_pool.tile([CD + 1, NB], FP32)
            nc.sync.dma_start(
                out=condT[0:CD, :],
                in_=bass.AP(tensor=cond_in, offset=0, ap=[[1, CD], [CD, NB]]),
            )
            nc.vector.memset(condT[CD:CD + 1, :], 1.0)
            aw2 = small_pool.tile([CD + 1, E], FP32)
            nc.sync.dma_start(out=aw2[0:CD, :], in_=aw_in[:, :])
            nc.sync.dma_start(
                out=aw2[CD:CD + 1, :],
                in_=bass.AP(tensor=ab_in, offset=0, ap=[[0, 1], [1, E]]),
            )
            p_log = psmall_pool.tile([NB, E], FP32, tag="ps")
            nc.tensor.matmul(p_log[:, :], condT[:, :], aw2[:, :], start=True, stop=True)
            aexp = small_pool.tile([NB, E], FP32)
            nc.scalar.activation(aexp[:, :], p_log[:, :], AF.Exp)
            asum = small_pool.tile([NB, 1], FP32)
            nc.vector.reduce_sum(out=asum[:, :], in_=aexp[:, :], axis=mybir.AxisListType.X)
            arec = small_pool.tile([NB, 1], FP32)
            nc.vector.reciprocal(arec[:, :], asum[:, :])
            alphas = small_pool.tile([NB, E], FP32)
            nc.scalar.mul(alphas[:, :], aexp[:, :], arec[:, 0:1])

            # broadcast alphas to all 128 partitions via DRAM round-trip
            adram = dram_pool.tile([NB, E], FP32)
            nc.sync.dma_start(out=adram[:, :], in_=aexp[:, :])
            a_bc = const_pool.tile([128, NB, E], FP32)
            adr_ap = adram[:, :]
            nc.gpsimd.dma_start(
                out=a_bc[:, :, :],
                in_=bass.AP(tensor=adr_ap.tensor, offset=adr_ap.offset,
                            ap=[[0, 128], [E, NB], [1, E]]),
            )

            ek_b = ek_pool.tile([128, E, KH, F], BF16)
            nc.gpsimd.dma_start(
                out=ek_b[:, :, :, :],
                in_=bass.AP(tensor=ek_in, offset=sKW,
                            ap=[[sKW, 2], [sC, Cin], [sE, E], [sKH, KH], [1, F]]),
            )
            ek_c = ek_pool.tile([128, E, KH, F], BF16)
            nc.gpsimd.dma_start(
                out=ek_c[0:Cin, :, :, :],
                in_=bass.AP(tensor=ek_in, offset=2 * sKW,
                            ap=[[sC, Cin], [sE, E], [sKH, KH], [1, F]]),
            )
            nc.gpsimd.dma_start(
                out=ek_c[Cin:128, :, :, :],
                in_=bass.AP(tensor=ek_in, offset=0,
                            ap=[[sC, Cin], [sE, E], [sKH, KH], [1, F]]),
            )
            nc.gpsimd.dma_start(out=xt0[:, 64:96, :], in_=x_in[0, :, 64:96, :])
            nc.gpsimd.dma_start(out=xt0[:, 96:128, :], in_=x_in[0, :, 96:128, :])

            # ---- mixed bias rows: biasT[b, f] = sum_e alphas[b,e] expert_bias[e,f]
            aT_ps = psmall_pool.tile([E, NB], FP32, tag="ps")
            nc.tensor.transpose(aT_ps[:, :], alphas[:, :], identE[0:E, 0:NB])
            aT_sb = small_pool.tile([E, NB], FP32)
            nc.vector.tensor_copy(aT_sb[:, :], aT_ps[:, :])
            eb_sb = small_pool.tile([E, F], FP32)
            nc.sync.dma_start(out=eb_sb[:, :], in_=eb_in[:, :])
            pbT = psmall_pool.tile([NB, F], FP32, tag="ps")
            nc.tensor.matmul(pbT[:, :], aT_sb[:, :], eb_sb[:, :], start=True, stop=True)
            biasT_sb = small_pool.tile([NB, F], FP32)
            nc.vector.tensor_copy(biasT_sb[:, :], pbT[:, :])
            # bias4[p, b, wl, f] = biasT[b, f] on every partition (DRAM trip)
            bdram = dram_pool.tile([NB, F], FP32)
            nc.sync.dma_start(out=bdram[:, :], in_=biasT_sb[:, :])
            rdram = dram_pool.tile([NB, 1], FP32)
            nc.sync.dma_start(out=rdram[:, :], in_=arec[:, :])
            bias4 = const_pool.tile([128, NB, 4, F], FP32)
            bdr_ap = bdram[:, :]
            for wl in range(4):
                nc.gpsimd.dma_start(
                    out=bias4[:, :, wl, :],
                    in_=bass.AP(tensor=bdr_ap.tensor, offset=bdr_ap.offset,
                                ap=[[0, 128], [F, NB], [1, F]]),
                )
            rec_bc = const_pool.tile([128, NB], FP32)
            rdr_ap = rdram[:, :]
            nc.gpsimd.dma_start(
                out=rec_bc[:, :],
                in_=bass.AP(tensor=rdr_ap.tensor, offset=rdr_ap.offset,
                            ap=[[0, 128], [1, NB]]),
            )

            # ---- per-sample weight mixing (fp32 accumulate, cast to bf16)
            mixed = {}

            def issue_mix(b):
                def alpha_ap(e):
                    return a_bc[:, b, e:e + 1]

                def mix(ek_stage, out_tile):
                    acc = mix_pool.tile([128, KH * F], FP32, tag="acc")
                    nc.scalar.mul(
                        acc[:, :],
                        ek_stage[:, 0, :, :].rearrange("p k f -> p (k f)"),
                        alpha_ap(0))
                    for e in range(1, E):
                        src = ek_stage[:, e, :, :].rearrange("p k f -> p (k f)")
                        dst = (acc[:, :] if e < E - 1
                               else out_tile[:, :, :].rearrange("p k f -> p (k f)"))
                        nc.vector.scalar_tensor_tensor(
                            out=dst, in0=src, scalar=alpha_ap(e), in1=acc[:, :],
                            op0=ALU.mult, op1=ALU.add)

                wa = w_pool.tile([128, KH, F], BF16, tag="wa")
                mix(ek_a, wa)
                wb_ = w_pool.tile([128, KH, F], BF16, tag="wb")
                mix(ek_b, wb_)
                wc = w_pool.tile([128, KH, F], BF16, tag="wc")
                mix(ek_c, wc)
                mixed[b] = (wa, wb_, wc)

            s_tiles = {}

            def build_s(b, kt_lo=0, kt_hi=W // 8, first=False):
                x_h = x_tiles[b]
                if kt_lo == 0:
                    s_t = s_pool.tile([128, NPK, HP], BF16, tag="s", name=f"s_t{b}")
                    nc.vector.memset(s_t[:, 0, :], 0.0)
                    nc.vector.memset(s_t[:, NPK - 1, :], 0.0)
                    nc.vector.memset(s_t[:, :, 0:1], 0.0)
                    nc.vector.memset(s_t[:, :, HP - 1:HP], 0.0)
                    s_tiles[b] = s_t
                s_t = s_tiles[b]
                for kt in range(kt_lo, kt_hi):
                    ptq = ptin_pool.tile([128, 4, H], BF16, tag="ptin")
                    for jj in range(4):
                        k = 4 * kt + jj
                        nc.tensor.matmul(
                            ptq[:, jj, :],
                            x_h[:, 2 * k:2 * k + 2, :].rearrange("h w c -> h (w c)"),
                            identb[:, :], is_transpose=True)
                    # ACT evacuates the transpose psum (DVE stays on conv evac)
                    nc.scalar.copy(s_t[:, 4 * kt + 1:4 * kt + 5, 1:H + 1],
                                   ptq[:, :, :])
                    if first and kt == 1:
                        issue_mix(b)

            build_s(0, 0, 8, first=True)

            for b in range(NB):
                wa, wb_, wc = mixed[b]
                s_t = s_tiles[b]
                sb2f = out_pool.tile([H, W, F], BF16, tag="sb2f")
                last = (b == NB - 1)
                for g in range(W // 4):
                    pk = 2 * g
                    pc = pconv_pool.tile([H, 4, F], FP32, tag="pc")
                    nmm = 0
                    for dh in range(KH):
                        for wl in range(4):
                            # col w = 4g + wl; even: full pair [W1;W2] on pair
                            # (k+1); odd: [W0;W1]. k = w//2.
                            wcol = 4 * g + wl
                            k = wcol // 2
                            full_w = wb_ if wcol % 2 == 0 else wa
                            nc.tensor.matmul(
                                pc[:, wl, :], s_t[:, k + 1, dh:dh + H],
                                full_w[:, dh, :],
                                start=(nmm == 0), stop=False)
                            nmm += 1
                            if wcol % 2 == 0:
                                # tap kw=0: col w-1 = bottom of pair k
                                nc.tensor.matmul(
                                    pc[:, wl, :], s_t[64:128, k, dh:dh + H],
                                    wc[64:128, dh, :],
                                    start=False, stop=(nmm == 23))
                            else:
                                # tap kw=2: col w+1 = top of pair k+2
                                nc.tensor.matmul(
                                    pc[:, wl, :], s_t[0:64, k + 2, dh:dh + H],
                                    wc[0:64, dh, :],
                                    start=False, stop=(nmm == 23))
                            nmm += 1
                    if g == 2 and b == 0:
                        build_s(0, 8, 16)
                    if g == 6 and b + 1 < NB:
                        load_x(b + 1)
                    if g == 16 and b + 1 < NB:
                        build_s(b + 1)
                    if g == 24 and b + 1 < NB:
                        issue_mix(b + 1)
                    # evacuate: fused softmax-normalize + bias + bf16 cast
                    nc.vector.scalar_tensor_tensor(
                        out=sb2f[:, 4 * g:4 * g + 4, :], in0=pc[:, :, :],
                        scalar=rec_bc[:, b:b + 1], in1=bias4[:, b, :, :],
                        op0=ALU.mult, op1=ALU.add)
                    if last and g >= 24:
                        nc.sync.dma_start(
                            out=out_t[b, :, 4 * g:4 * g + 4, :],
                            in_=sb2f[:, 4 * g:4 * g + 4, :])
                    elif g % 4 == 3 and (not last or g < 24):
                        q = g // 4
                        nc.sync.dma_start(
                            out=out_t[b, :, 16 * q:16 * q + 16, :],
                            in_=sb2f[:, 16 * q:16 * q + 16, :])
    nc.compile()
    return nc


def kernel(x, cond, alpha_w, alpha_b, expert_kernels, expert_bias, trace=False):
    if "nc" not in _cache:
        _cache["nc"] = _build_nc()
    nc = _cache["nc"]
    aw = np.ascontiguousarray(np.asarray(alpha_w, dtype=np.float32))
    ab = np.ascontiguousarray(np.asarray(alpha_b, dtype=np.float32))
    ek = np.ascontiguousarray(np.asarray(expert_kernels, dtype=np.float32))
    eb = np.ascontiguousarray(np.asarray(expert_bias, dtype=np.float32))
    x = np.asarray(x, dtype=np.float32)
    cond = np.asarray(cond, dtype=np.float32)
    in_maps = []
    for c in range(NCORES):
        in_maps.append({
            "x": np.ascontiguousarray(x[c * NB:(c + 1) * NB]),
            "cond": np.ascontiguousarray(cond[c * NB:(c + 1) * NB]),
            "alpha_w": aw, "alpha_b": ab,
            "expert_kernels": ek, "expert_bias": eb,
        })
    res = run_bass_kernel_spmd(nc, in_maps, core_ids=list(range(NCORES)), trace=trace)
    _cache["last_result"] = res
    return np.concatenate(
        [np.asarray(r["out"], dtype=np.float32) for r in res.results], axis=0)


# revision 44
# speedup vs baseline: 1.6243x; 1.0021x over previous
"""CondConv2D Trainium2 kernel: data-parallel over batch across 8 NeuronCores.

Per core (4 samples):
  1. alphas = softmax(cond @ alpha_w + alpha_b)   [alpha_b folded into the
     matmul via an appended ones-row, tiny PE matmul + ACT/DVE softmax]
  2. K_mix[b] = sum_e alphas[b,e] * expert_kernels[e]
  3. conv2d(x[b], K_mix[b], SAME) + bias_mix[b]

Conv strategy (x-stationary, h-major output; non-overlapping column pairs):
  x[b] is SWDGE-cast-loaded as bf16 in natural [h, (w,c)] layout (sample 0
  in 4 w-chunks so PE transposes start early). 64 PE transposes of
  disjoint 2-column blocks build S[(c,2), pk, hp] bf16 where pair pk holds
  cols (2pk-2, 2pk-1); pk=0/65 zero pads, hp pads rows (ACT evacuates the
  transpose PSUM, DVE memsets the pads). Conv matmuls put the S-patch as
  the stationary operand and mixed weights as the moving operand, so
  output lands as [h, (w,F)] in PSUM — already HBM-ordered, no output
  transpose. Per group of 4 output columns one PSUM bank [H, 4, F] fp32
  accumulates 24 matmuls (N=F): per column per kh, one full-K pair matmul
  (even cols [W1;W2], odd cols [W0;W1]) + one K=64 edge matmul (W0 on pair
  bottoms / W2 on pair tops). Weights are mixed with UNNORMALIZED
  exp-logits; the softmax 1/sum rides the evacuation: a single DVE
  scalar_tensor_tensor fuses the normalize, broadcast-bias add, and bf16
  cast while evacuating into a per-sample [H, W, F] buffer, stored to HBM
  in 4KB-run chunks. Output HBM tensor is bf16; the host casts back to
  fp32. Expert kernels stage as bf16 (SWDGE cast DMA).
"""

import numpy as np

import concourse.bass as bass
import concourse.bacc as bacc
import concourse.mybir as mybir
import concourse.tile as tile
from concourse.bass_utils import run_bass_kernel_spmd
from concourse.masks import make_identity

B, H, W, Cin, E, F = 32, 128, 128, 64, 4, 128
KH = KW = 3
NCORES = 8
NB = B // NCORES  # 4 samples per core
CD = 64  # cond dim
HP = H + 2  # padded row index j; row = j-1
NPK = W // 2 + 2  # 66 pairs; pair pk = cols (2pk-2, 2pk-1); pk 0 and 65 zero

FP32 = mybir.dt.float32
BF16 = mybir.dt.bfloat16
AF = mybir.ActivationFunctionType
ALU = mybir.AluOpType

_cache = {}


def _build_nc():
    nc = bacc.Bacc(None)
    x_in = nc.dram_tensor("x", [NB, H, W, Cin], FP32, kind="ExternalInput")
    cond_in = nc.dram_tensor("cond", [NB, CD], FP32, kind="ExternalInput")
    aw_in = nc.dram_tensor("alpha_w", [CD, E], FP32, kind="ExternalInput")
    ab_in = nc.dram_tensor("alpha_b", [E], FP32, kind="ExternalInput")
    ek_in = nc.dram_tensor("expert_kernels", [E, KH, KW, Cin, F], FP32, kind="ExternalInput")
    eb_in = nc.dram_tensor("expert_bias", [E, F], FP32, kind="ExternalInput")
    out_t = nc.dram_tensor("out", [NB, H, W, F], BF16, kind="ExternalOutput")

    with tile.TileContext(nc) as tc:
        with (
            tc.tile_pool(name="const", bufs=1) as const_pool,
            tc.tile_pool(name="ek", bufs=1) as ek_pool,
            tc.tile_pool(name="mix", bufs=2) as mix_pool,
            tc.tile_pool(name="wts", bufs=2) as w_pool,
            tc.tile_pool(name="xin", bufs=2) as x_pool,
            tc.tile_pool(name="stk", bufs=3) as s_pool,
            tc.tile_pool(name="outb", bufs=2) as out_pool,
            tc.tile_pool(name="small", bufs=2) as small_pool,
            tc.tile_pool(name="dram", bufs=1, space="DRAM") as dram_pool,
            tc.tile_pool(name="pconv", bufs=4, space="PSUM") as pconv_pool,
            tc.tile_pool(name="ptin", bufs=3, space="PSUM") as ptin_pool,
            tc.tile_pool(name="psmall", bufs=1, space="PSUM") as psmall_pool,
        ):
            # identity first: Pool op gating the PE transposes
            identb = const_pool.tile([128, 128], BF16)
            make_identity(nc, identb[:, :])
            identE = const_pool.tile([NB, NB], FP32)
            make_identity(nc, identE[:, :])

            sE, sKH, sKW, sC = KH * KW * Cin * F, KW * Cin * F, Cin * F, F
            x_tiles = {}

            def load_x(b, chunks=1):
                xt = x_pool.tile([H, W, Cin], BF16, tag="x", name=f"x_h{b}")
                wq = W // chunks
                for qc in range(chunks):
                    nc.gpsimd.dma_start(
                        out=xt[:, wq * qc:wq * qc + wq, :],
                        in_=x_in[b, :, wq * qc:wq * qc + wq, :])
                x_tiles[b] = xt

            # Pool/SWDGE issue order drives the serial DMA schedule:
            # x0c0, ek_a, ek_b, ek_c, x0c1, a_bc, bias4, x0c2, x0c3, x1
            xt0 = x_pool.tile([H, W, Cin], BF16, tag="x", name="x_h0")
            x_tiles[0] = xt0
            nc.gpsimd.dma_start(out=xt0[:, 0:32, :], in_=x_in[0, :, 0:32, :])
            nc.gpsimd.dma_start(out=xt0[:, 32:64, :], in_=x_in[0, :, 32:64, :])

            # expert kernel staging (bf16 via SWDGE cast; 512B source runs)
            ek_a = ek_pool.tile([128, E, KH, F], BF16)
            nc.gpsimd.dma_start(
                out=ek_a[:, :, :, :],
                in_=bass.AP(tensor=ek_in, offset=0,
                            ap=[[sKW, 2], [sC, Cin], [sE, E], [sKH, KH], [1, F]]),
            )
            # ---- routing: alphas = softmax([cond, 1] @ [alpha_w; alpha_b])
            condT = small_pool.tile([CD + 1, NB], FP32)
            nc.sync.dma_start(
                out=condT[0:CD, :],
                in_=bass.AP(tensor=cond_in, offset=0, ap=[[1, CD], [CD, NB]]),
            )
            nc.vector.memset(condT[CD:CD + 1, :], 1.0)
            aw2 = small_pool.tile([CD + 1, E], FP32)
            nc.sync.dma_start(out=aw2[0:CD, :], in_=aw_in[:, :])
            nc.sync.dma_start(
                out=aw2[CD:CD + 1, :],
                in_=bass.AP(tensor=ab_in, offset=0, ap=[[0, 1], [1, E]]),
            )
            p_log = psmall_pool.tile([NB, E], FP32, tag="ps")
            nc.tensor.matmul(p_log[:, :], condT[:, :], aw2[:, :], start=True, stop=True)
            aexp = small_pool.tile([NB, E], FP32)
            nc.scalar.activation(aexp[:, :], p_log[:, :], AF.Exp)
            asum = small_pool.tile([NB, 1], FP32)
            nc.vector.reduce_sum(out=asum[:, :], in_=aexp[:, :], axis=mybir.AxisListType.X)
            arec = small_pool.tile([NB, 1], FP32)
            nc.vector.reciprocal(arec[:, :], asum[:, :])
            alphas = small_pool.tile([NB, E], FP32)
            nc.scalar.mul(alphas[:, :], aexp[:, :], arec[:, 0:1])

            # broadcast alphas to all 128 partitions via DRAM round-trip
            adram = dram_pool.tile([NB, E], FP32)
            nc.sync.dma_start(out=adram[:, :], in_=aexp[:, :])
            a_bc = const_pool.tile([128, NB, E], FP32)
            adr_ap = adram[:, :]
            nc.gpsimd.dma_start(
                out=a_bc[:, :, :],
                in_=bass.AP(tensor=adr_ap.tensor, offset=adr_ap.offset,
                            ap=[[0, 128], [E, NB], [1, E]]),
            )

            ek_b = ek_pool.tile([128, E, KH, F], BF16)
            nc.gpsimd.dma_start(
                out=ek_b[:, :, :, :],
                in_=bass.AP(tensor=ek_in, offset=sKW,
                            ap=[[sKW, 2], [sC, Cin], [sE, E], [sKH, KH], [1, F]]),
            )
            ek_c = ek_pool.tile([128, E, KH, F], BF16)
            nc.gpsimd.dma_start(
                out=ek_c[0:Cin, :, :, :],
                in_=bass.AP(tensor=ek_in, offset=2 * sKW,
                            ap=[[sC, Cin], [sE, E], [sKH, KH], [1, F]]),
            )
            nc.gpsimd.dma_start(
                out=ek_c[Cin:128, :, :, :],
                in_=bass.AP(tensor=ek_in, offset=0,
                            ap=[[sC, Cin], [sE, E], [sKH, KH], [1, F]]),
            )
            nc.gpsimd.dma_start(out=xt0[:, 64:96, :], in_=x_in[0, :, 64:96, :])
            nc.gpsimd.dma_start(out=xt0[:, 96:128, :], in_=x_in[0, :, 96:128, :])

            # ---- mixed bias rows: biasT[b, f] = sum_e alphas[b,e] expert_bias[e,f]
            aT_ps = psmall_pool.tile([E, NB], FP32, tag="ps")
            nc.tensor.transpose(aT_ps[:, :], alphas[:, :], identE[0:E, 0:NB])
            aT_sb = small_pool.tile([E, NB], FP32)
            nc.vector.tensor_copy(aT_sb[:, :], aT_ps[:, :])
            eb_sb = small_pool.tile([E, F], FP32)
            nc.sync.dma_start(out=eb_sb[:, :], in_=eb_in[:, :])
            pbT = psmall_pool.tile([NB, F], FP32, tag="ps")
            nc.tensor.matmul(pbT[:, :], aT_sb[:, :], eb_sb[:, :], start=True, stop=True)
            biasT_sb = small_pool.tile([NB, F], FP32)
            nc.vector.tensor_copy(biasT_sb[:, :], pbT[:, :])
            # bias4[p, b, wl, f] = biasT[b, f] on every partition (DRAM trip)
            bdram = dram_pool.tile([NB, F], FP32)
            nc.sync.dma_start(out=bdram[:, :], in_=biasT_sb[:, :])
            rdram = dram_pool.tile([NB, 1], FP32)
            nc.sync.dma_start(out=rdram[:, :], in_=arec[:, :])
            bias4 = const_pool.tile([128, NB, 4, F], FP32)
            bdr_ap = bdram[:, :]
            for wl in range(4):
                nc.gpsimd.dma_start(
                    out=bias4[:, :, wl, :],
                    in_=bass.AP(tensor=bdr_ap.tensor, offset=bdr_ap.offset,
                                ap=[[0, 128], [F, NB], [1, F]]),
                )
            rec_bc = const_pool.tile([128, NB], FP32)
            rdr_ap = rdram[:, :]
            nc.gpsimd.dma_start(
                out=rec_bc[:, :],
                in_=bass.AP(tensor=rdr_ap.tensor, offset=rdr_ap.offset,
                            ap=[[0, 128], [1, NB]]),
            )

            # ---- per-sample weight mixing (fp32 accumulate, cast to bf16)
            mixed = {}

            def issue_mix(b):
                def alpha_ap(e):
                    return a_bc[:, b, e:e + 1]

                def mix(ek_stage, out_tile):
                    acc = mix_pool.tile([128, KH * F], FP32, tag="acc")
                    nc.scalar.mul(
                        acc[:, :],
                        ek_stage[:, 0, :, :].rearrange("p k f -> p (k f)"),
                        alpha_ap(0))
                    for e in range(1, E):
                        src = ek_stage[:, e, :, :].rearrange("p k f -> p (k f)")
                        dst = (acc[:, :] if e < E - 1
                               else out_tile[:, :, :].rearrange("p k f -> p (k f)"))
                        nc.vector.scalar_tensor_tensor(
                            out=dst, in0=src, scalar=alpha_ap(e), in1=acc[:, :],
                            op0=ALU.mult, op1=ALU.add)

                wa = w_pool.tile([128, KH, F], BF16, tag="wa")
                mix(ek_a, wa)
                wb_ = w_pool.tile([128, KH, F], BF16, tag="wb")
                mix(ek_b, wb_)
                wc = w_pool.tile([128, KH, F], BF16, tag="wc")
                mix(ek_c, wc)
                mixed[b] = (wa, wb_, wc)

            s_tiles = {}

            def build_s(b, kt_lo=0, kt_hi=W // 8, first=False):
                x_h = x_tiles[b]
                if kt_lo == 0:
                    s_t = s_pool.tile([128, NPK, HP], BF16, tag="s", name=f"s_t{b}")
                    nc.vector.memset(s_t[:, 0, :], 0.0)
                    nc.vector.memset(s_t[:, NPK - 1, :], 0.0)
                    nc.vector.memset(s_t[:, :, 0:1], 0.0)
                    nc.vector.memset(s_t[:, :, HP - 1:HP], 0.0)
                    s_tiles[b] = s_t
                s_t = s_tiles[b]
                for kt in range(kt_lo, kt_hi):
                    ptq = ptin_pool.tile([128, 4, H], BF16, tag="ptin")
                    for jj in range(4):
                        k = 4 * kt + jj
                        nc.tensor.matmul(
                            ptq[:, jj, :],
                            x_h[:, 2 * k:2 * k + 2, :].rearrange("h w c -> h (w c)"),
                            identb[:, :], is_transpose=True)
                    # ACT evacuates the transpose psum (DVE stays on conv evac)
                    nc.scalar.copy(s_t[:, 4 * kt + 1:4 * kt + 5, 1:H + 1],
                                   ptq[:, :, :])
                    if first and kt == 1:
                        issue_mix(b)

            build_s(0, 0, 8, first=True)

            for b in range(NB):
                wa, wb_, wc = mixed[b]
                s_t = s_tiles[b]
                sb2f = out_pool.tile([H, W, F], BF16, tag="sb2f")
                last = (b == NB - 1)
                for g in range(W // 4):
                    pk = 2 * g
                    pc = pconv_pool.tile([H, 4, F], FP32, tag="pc")
                    nmm = 0
                    for dh in range(KH):
                        for wl in range(4):
                            # col w = 4g + wl; even: full pair [W1;W2] on pair
                            # (k+1); odd: [W0;W1]. k = w//2.
                            wcol = 4 * g + wl
                            k = wcol // 2
                            full_w = wb_ if wcol % 2 == 0 else wa
                            nc.tensor.matmul(
                                pc[:, wl, :], s_t[:, k + 1, dh:dh + H],
                                full_w[:, dh, :],
                                start=(nmm == 0), stop=False)
                            nmm += 1
                            if wcol % 2 == 0:
                                # tap kw=0: col w-1 = bottom of pair k
                                nc.tensor.matmul(
                                    pc[:, wl, :], s_t[64:128, k, dh:dh + H],
                                    wc[64:128, dh, :],
                                    start=False, stop=(nmm == 23))
                            else:
                                # tap kw=2: col w+1 = top of pair k+2
                                nc.tensor.matmul(
                                    pc[:, wl, :], s_t[0:64, k + 2, dh:dh + H],
                                    wc[0:64, dh, :],
                                    start=False, stop=(nmm == 23))
                            nmm += 1
                    if g == 2 and b == 0:
                        build_s(0, 8, 16)
                    if g == 6 and b + 1 < NB:
                        load_x(b + 1)
                    if g == 16 and b + 1 < NB:
                        build_s(b + 1)
                    if g == 24 and b + 1 < NB:
                        issue_mix(b + 1)
                    # evacuate: fused softmax-normalize + bias + bf16 cast
                    nc.vector.scalar_tensor_tensor(
                        out=sb2f[:, 4 * g:4 * g + 4, :], in0=pc[:, :, :],
                        scalar=rec_bc[:, b:b + 1], in1=bias4[:, b, :, :],
                        op0=ALU.mult, op1=ALU.add)
                    if last and g >= 24:
                        nc.sync.dma_start(
                            out=out_t[b, :, 4 * g:4 * g + 4, :],
                            in_=sb2f[:, 4 * g:4 * g + 4, :])
                    elif g % 4 == 3 and (not last or g < 24):
                        q = g // 4
                        nc.sync.dma_start(
                            out=out_t[b, :, 16 * q:16 * q + 16, :],
                            in_=sb2f[:, 16 * q:16 * q + 16, :])
    nc.compile()
    return nc


def kernel(x, cond, alpha_w, alpha_b, expert_kernels, expert_bias, trace=False):
    if "nc" not in _cache:
        _cache["nc"] = _build_nc()
    nc = _cache["nc"]
    aw = np.ascontiguousarray(np.asarray(alpha_w, dtype=np.float32))
    ab = np.ascontiguousarray(np.asarray(alpha_b, dtype=np.float32))
    ek = np.ascontiguousarray(np.asarray(expert_kernels, dtype=np.float32))
    eb = np.ascontiguousarray(np.asarray(expert_bias, dtype=np.float32))
    x = np.asarray(x, dtype=np.float32)
    cond = np.asarray(cond, dtype=np.float32)
    in_maps = []
    for c in range(NCORES):
        in_maps.append({
            "x": np.ascontiguousarray(x[c * NB:(c + 1) * NB]),
            "cond": np.ascontiguousarray(cond[c * NB:(c + 1) * NB]),
            "alpha_w": aw, "alpha_b": ab,
            "expert_kernels": ek, "expert_bias": eb,
        })
    res = run_bass_kernel_spmd(nc, in_maps, core_ids=list(range(NCORES)), trace=trace)
    _cache["last_result"] = res
    return np.concatenate(
        [np.asarray(r["out"], dtype=np.float32) for r in res.results], axis=0)
